# revision 25
# baseline (speedup 1.0000x reference)
"""Trainium2 Bass kernel for nn_AddPoolingFusion.

Reference computation (b=16, l1=l2=2048, d1=d2=d3=768):
    y1  = x1 @ W1.T + b1                      # [b, l1, d3]
    y2  = x2 @ W2.T + b2                      # [b, l2, d3]
    out = y1 + mean(y2, axis=1, keepdims=True)

Because the mean over l2 commutes with the linear layer:
    out[b,i,:] = x1[b,i] @ W1.T + c[b]
    c[b]       = (b1 + b2) + mean_j(x2[b,j]) @ W2.T

Strategy: data-parallel over batch, 2 batches per core, no collectives.
The per-core floor is the x1 matmul on TensorE: 32 m-tiles x 6 k-chunks
x (512+256) columns = 147456 PE cycles ~= 74us at the 2.0 GHz sustained
(P0) clock. Everything else is scheduled around keeping that stream
dense from ~3us to the end:

- DMA priority: W1 (split; first chunks first) and x1 (16 quarter-group
  transfers on the Sync HWDGE ring) lead; x2/W2 are held back by an
  explicit dep so they never steal HBM bandwidth from the x1 stream.
- Warm-up matmuls on junk data run during the initial DMA fill so the
  PE's HAM activity window flips to full clock before real work lands.
- x2 is pre-transposed on the host to [d2-partition, l2-free] so the
  per-batch mean is a free-dim reduction, done on the otherwise-idle
  Scalar engine (activation copy with accum_out). This removes all
  partition-reduce matmuls from the PE stream.
- c[b] = xbar2 @ W2.T for BOTH batches in one 12-matmul group (the two
  means are the 2 columns of the stationary operand), inserted mid-way
  through group 2 where its inputs are long ready.
- Output groups 0-2 evacuate PSUM as plain copies (bias deferred until
  c is ready); the deferred bias adds + stores are interleaved into
  group 3's emission so the DVE never head-blocks the PSUM evacuations.

Host-side prep (layout/dtype only): partition-major SBUF images, bf16
casts (rel err ~3e-3 vs the 2e-2 gate), output stored bf16 and upcast
on the host.
"""

import sys

import numpy as np

# concourse normally comes from the axon site overlay already on sys.path;
# append /opt/trn_rl_repo as a fallback only.
if "/opt/trn_rl_repo" not in sys.path:
    sys.path.append("/opt/trn_rl_repo")

N_CORES = 8
B_PER_CORE = 2
L = 2048
D = 768  # d1 == d2 == d3 == 768
P = 128
NCH = D // P  # 6 contraction chunks
M = B_PER_CORE * L  # 4096 rows per core
NQ = 16  # x1 quarter-group DMAs, 2 m-tiles each
TPG = 8  # m-tiles per output group
NGRP = (M // P) // TPG  # 4 groups
SPT = 4  # m-tiles per output store
NST = (M // P) // SPT  # 8 stores


def build_nc(debug=False, explicit_ldw=False):
    import concourse.bacc as bacc
    import concourse.mybir as mybir
    import concourse.tile as tile

    f32 = mybir.dt.float32
    bf16 = mybir.dt.bfloat16
    fp8 = mybir.dt.float8e4
    add = mybir.AluOpType.add
    Copy = mybir.ActivationFunctionType.Copy

    nc = bacc.Bacc(None, target_bir_lowering=False, debug=debug)

    # x2 and W2 only feed the mean term c[b] (magnitude ~5% of the output,
    # averaged over 2048 rows), so fp8 quantization error lands ~1e-3 on the
    # output while halving their HBM traffic - the load phase is what gates
    # the c-path readiness.
    x1h = nc.declare_dram_parameter("x1h", [NQ, P, NCH, 2 * P], bf16, isOutput=False)
    x2h = nc.declare_dram_parameter("x2h", [B_PER_CORE, P, NCH, L], fp8, isOutput=False)
    w1h = nc.declare_dram_parameter("w1h", [P, NCH, D], bf16, isOutput=False)
    w2h = nc.declare_dram_parameter("w2h", [P, NCH, D], fp8, isOutput=False)
    bsh = nc.declare_dram_parameter("bsh", [B_PER_CORE, D], f32, isOutput=False)
    outh = nc.declare_dram_parameter("outh", [NST, P, SPT, D], bf16, isOutput=True)

    with tile.TileContext(nc) as tc:
        with (
            tc.tile_pool(name="const", bufs=1) as const,
            tc.tile_pool(name="x1p", bufs=1) as x1p,
            tc.tile_pool(name="x2p", bufs=1) as x2p,
            tc.tile_pool(name="yp", bufs=1) as yp,
            tc.tile_pool(name="psY", bufs=3, space="PSUM") as psY,
            tc.tile_pool(name="psC", bufs=1, space="PSUM") as psC,
        ):
            # ---- early constants ----
            warm = const.tile([P, 512], bf16)
            nc.vector.memset(warm[:], 0.03125)
            w1sb = const.tile([P, NCH, D], bf16)
            # W1 in 3 chunk pairs so the first matmuls' weights land first
            nc.scalar.dma_start(w1sb[:, 0:2, :], w1h[:, 0:2, :])
            nc.scalar.dma_start(w1sb[:, 2:4, :], w1h[:, 2:4, :])
            nc.scalar.dma_start(w1sb[:, 4:6, :], w1h[:, 4:6, :])

            # ---- x1 quarter-group loads (Sync ring), 3 in flight ----
            # The SDMA engines round-robin over ALL queued descriptors at
            # packet granularity, so an unchained backlog makes every
            # transfer finish near the end together; a depth-3 chain keeps
            # completions incremental at full ring throughput.
            x1tiles = []
            x1dmas = []
            for q in range(NQ):
                xq = x1p.tile([P, NCH, 2 * P], bf16, tag=f"x1q{q}")
                x1tiles.append(xq)
                dma = nc.sync.dma_start(xq[:], x1h[q])
                if q >= 3:
                    tile.add_dep_helper(
                        dma.ins, x1dmas[q - 3].ins, sync=True,
                        reason="x1 JIT pacing",
                    )
                x1dmas.append(dma)

            # ---- x2 chunk loads + mean, both on the Scalar engine ----
            # xbtf[p, c, b] = sum_j x2[b, j, c*128+p] via activation with
            # accum_out. Emission interleaves [reduce chunk i, dma chunk
            # i+3]: each 2.2us reduce paces the next chunk dispatch, so ~3
            # chunks stay in flight with no sequencer-stalling dep chain,
            # and the reduces start as soon as the first chunk lands.
            x2tiles = []
            x2dmas = []
            bsum_sb = const.tile([B_PER_CORE, D], f32)
            for b in range(B_PER_CORE):
                xt = x2p.tile([P, NCH, L], fp8, tag=f"x2t{b}")
                x2tiles.append(xt)

            def x2_dma(i):
                b, c = i // NCH, i % NCH
                dma = nc.scalar.dma_start(
                    x2tiles[b][:, c, :], x2h[b, :, c, :]
                )
                if i == 0:
                    # x2 yields HBM bandwidth until x1 quarter 0 has landed
                    tile.add_dep_helper(
                        dma.ins, x1dmas[0].ins, sync=True,
                        reason="x2 yields to x1 q0",
                    )
                x2dmas.append(dma)

            xbtf = const.tile([P, NCH, B_PER_CORE], f32)
            scr = const.tile([P, L], bf16)
            for i in range(3):
                x2_dma(i)
            for i in range(B_PER_CORE * NCH):
                b, c = i // NCH, i % NCH
                nc.scalar.activation(
                    scr[:], x2tiles[b][:, c, :], Copy,
                    accum_out=xbtf[:, c, b : b + 1],
                )
                if i + 3 < B_PER_CORE * NCH:
                    x2_dma(i + 3)
                if i == NCH - 1:
                    nc.scalar.dma_start(bsum_sb[:], bsh[:])
                    w2sb = const.tile([P, NCH, D], fp8)
                    nc.scalar.dma_start(w2sb[:], w2h[:])

            # ---- warm-up matmuls: flip the HAM clock gate during DMA fill ----
            pc = psC.tile([1, D], f32)
            for _ in range(3):
                nc.tensor.matmul(
                    pc[:, 0:512], warm[:, 0:1], warm[:], start=True, stop=True,
                )

            # ---- main matmul stream ----
            cr = [None, None]

            def emit_mtile(t, ys, fused_b=None):
                q, sub = t // 2, t % 2
                xq = x1tiles[q]
                py_ = psY.tile([P, D], f32)
                for c in range(NCH):
                    xw = xq[:, c, sub * P : (sub + 1) * P]
                    nc.tensor.matmul(
                        py_[:, 0:512], xw, w1sb[:, c, 0:512],
                        start=(c == 0), stop=(c == NCH - 1),
                    )
                    nc.tensor.matmul(
                        py_[:, 512:768], xw, w1sb[:, c, 512:768],
                        start=(c == 0), stop=(c == NCH - 1),
                    )
                tl = t % TPG
                if fused_b is None:
                    nc.vector.tensor_copy(ys[:, tl, :], py_[:])
                else:
                    nc.vector.tensor_tensor(
                        ys[:, tl, :], py_[:], cr[fused_b][:], op=add
                    )

            def group_tiles(g):
                ys = yp.tile([P, TPG, D], bf16, tag=f"ys{g}")
                return ys

            ys_g = [group_tiles(g) for g in range(NGRP)]

            # t0..t15: deferred bias (c is not ready yet)
            with nc.named_scope("grp01"):
                for t in range(0, 16):
                    emit_mtile(t, ys_g[t // TPG])

            # ---- c path: c[b] = mean(x2[b]) @ W2.T + (b1 + b2) ----
            # Two M=1 matmul passes sharing one PSUM tile, so both results
            # land on partition 0 where partition_broadcast can source
            # them directly (an SBUF->SBUF hop for partition 1 costs ~4us
            # of serial DMA latency). Batch 1's pass runs right after t15
            # so cr1 is ready when t16's fused evacuation is due; batch
            # 0's pass is deferred past t17 to spread the PE cost.
            xbtb = const.tile([P, NCH, B_PER_CORE], bf16)

            def c_lin(b):
                for c in range(NCH):
                    nc.tensor.matmul(
                        pc[:, 0:512], xbtb[:, c, b : b + 1],
                        w2sb[:, c, 0:512],
                        start=(c == 0), stop=(c == NCH - 1),
                    )
                for c in range(NCH):
                    nc.tensor.matmul(
                        pc[:, 512:768], xbtb[:, c, b : b + 1],
                        w2sb[:, c, 512:768],
                        start=(c == 0), stop=(c == NCH - 1),
                    )
                csb = const.tile([1, D], bf16, tag=f"cs{b}")
                nc.vector.tensor_tensor(csb[:], pc[:], bsum_sb[0:1, :], op=add)
                crb = const.tile([P, D], bf16, tag=f"cr{b}")
                nc.gpsimd.partition_broadcast(crb[:], csb[:])
                cr[b] = crb

            with nc.named_scope("c_path_b1"):
                # mean scale + bf16 cast on idle GpSimd, off the DVE FIFO
                nc.gpsimd.tensor_scalar_mul(xbtb[:], xbtf[:], 1.0 / L)
                c_lin(1)

            # deferred bias + stores for t0..t15, interleaved into the
            # t16..t31 emission at <=2 per m-tile so the DVE keeps pace
            # with the PSUM evacuations (evac 0.9us + 2 adds ~1us vs the
            # 1.95us m-tile period)
            flush_items = []
            for g in (1, 0):  # group 1 first: ys and cr0 ready earliest
                for tl in range(TPG):
                    flush_items.append(("add", g, tl))
                    if tl % SPT == SPT - 1:
                        flush_items.append(("store", g, tl))

            def emit_flush(item):
                if item[0] == "add":
                    # group 0's adds run on the idle GpSimd so the DVE
                    # stays under the m-tile period (evac + 1 add) and
                    # never backpressures the PE through PSUM
                    _, g, tl = item
                    eng = nc.gpsimd if g == 0 else nc.vector
                    eng.tensor_tensor(
                        ys_g[g][:, tl, :], ys_g[g][:, tl, :], cr[0][:], op=add
                    )
                else:
                    # deferred stores drain on the Scalar ring, which is
                    # idle after the loads - the Sync ring keeps the
                    # fused-path stores, halving the store-drain time
                    _, g, tl = item
                    st = g * 2 + tl // SPT
                    nc.scalar.dma_start(
                        outh[st], ys_g[g][:, tl - SPT + 1 : tl + 1, :]
                    )

            slot_quota = {t: (1 if t < 26 else 3) for t in range(18, 32)}
            fi = 0
            with nc.named_scope("grp23"):
                for t in range(16, 32):
                    g = t // TPG
                    emit_mtile(t, ys_g[g], fused_b=1)
                    if t == 17:
                        with nc.named_scope("c_path_b0"):
                            c_lin(0)
                    tl = t % TPG
                    if t < 28:
                        if tl % SPT == SPT - 1:
                            nc.sync.dma_start(
                                outh[t // SPT],
                                ys_g[g][:, tl - SPT + 1 : tl + 1, :],
                            )
                    else:
                        # final tiles: single-m-tile stores alternating
                        # across both rings so the tail drains in parallel
                        ring = nc.sync if t % 2 == 0 else nc.scalar
                        ring.dma_start(
                            outh[7][:, tl - SPT : tl - SPT + 1, :],
                            ys_g[3][:, tl : tl + 1, :],
                        )
                    for _ in range(slot_quota.get(t, 0)):
                        if fi < len(flush_items):
                            emit_flush(flush_items[fi])
                            fi += 1
                while fi < len(flush_items):
                    emit_flush(flush_items[fi])
                    fi += 1

    return nc


def make_in_maps(x1, x2, W1, b1, W2, b2):
    import ml_dtypes

    bf16 = ml_dtypes.bfloat16
    fp8 = ml_dtypes.float8_e4m3fn

    def wlayout(W, dt):
        # [e, d] -> W.T [d, e] -> [p, c, e] with d = c*128 + p
        wt = np.ascontiguousarray(W.T).reshape(NCH, P, D).transpose(1, 0, 2)
        return np.ascontiguousarray(wt).astype(dt)

    w1h = wlayout(W1, bf16)
    w2h = wlayout(W2, fp8)
    bsh = np.ascontiguousarray(
        np.broadcast_to((b1 + b2).reshape(1, D), (B_PER_CORE, D))
    ).astype(np.float32)
    in_maps = []
    for k in range(N_CORES):
        x1_s = x1[k * B_PER_CORE : (k + 1) * B_PER_CORE]  # [2, 2048, 768]
        x2_s = x2[k * B_PER_CORE : (k + 1) * B_PER_CORE]
        # x1t [d, m] with col m = b*2048 + i, then quarter-major image
        x1t = np.transpose(x1_s, (2, 0, 1)).reshape(D, M)
        x1h = np.ascontiguousarray(
            x1t.reshape(NCH, P, NQ, 2 * P).transpose(2, 1, 0, 3)
        ).astype(bf16)  # [q, p, c, m_in_quarter]
        # x2 transposed: [b, p, c, j] with d = c*128 + p
        x2h = np.ascontiguousarray(
            np.transpose(x2_s, (0, 2, 1)).reshape(B_PER_CORE, NCH, P, L)
            .transpose(0, 2, 1, 3)
        ).astype(fp8)
        in_maps.append(
            {"x1h": x1h, "x2h": x2h, "w1h": w1h, "w2h": w2h, "bsh": bsh}
        )
    return in_maps


def kernel(x1, x2, W1, b1, W2, b2, trace=False, explicit_ldw=False):
    from concourse.bass_utils import run_bass_kernel_spmd

    # accept jax arrays / lists transparently
    x1, x2, W1, b1, W2, b2 = (
        np.asarray(t, dtype=np.float32) for t in (x1, x2, W1, b1, W2, b2)
    )
    nc = build_nc(debug=False, explicit_ldw=explicit_ldw)
    nc.finalize()
    in_maps = make_in_maps(x1, x2, W1, b1, W2, b2)
    res = run_bass_kernel_spmd(
        nc, in_maps, core_ids=list(range(N_CORES)), trace=trace
    )
    shards = []
    for k in range(N_CORES):
        oh = res.results[k]["outh"]  # [NST, P, SPT, D] bf16, row = (s*SPT+t)*128+p
        flat = (
            oh.astype(np.float32).transpose(0, 2, 1, 3).reshape(M, D)
        )
        shards.append(flat.reshape(B_PER_CORE, L, D))
    out = np.concatenate(shards, axis=0)
    if trace:
        kernel.last_result = res
    return out


# revision 26
# speedup vs baseline: 1.0375x; 1.0375x over previous
"""Trainium2 Bass kernel for nn_AddPoolingFusion.

Reference computation (b=16, l1=l2=2048, d1=d2=d3=768):
    y1  = x1 @ W1.T + b1                      # [b, l1, d3]
    y2  = x2 @ W2.T + b2                      # [b, l2, d3]
    out = y1 + mean(y2, axis=1, keepdims=True)

Because the mean over l2 commutes with the linear layer:
    out[b,i,:] = x1[b,i] @ W1.T + c[b]
    c[b]       = (b1 + b2) + mean_j(x2[b,j]) @ W2.T

Strategy: data-parallel over batch, 2 batches per core, no collectives.
The per-core floor is the x1 matmul on TensorE: 32 m-tiles x 6 k-chunks
x (512+256) columns = 147456 PE cycles ~= 74us at the 2.0 GHz sustained
(P0) clock. Everything else is scheduled around keeping that stream
dense from ~3us to the end:

- DMA priority: W1 (split; first chunks first) and x1 (16 quarter-group
  transfers on the Sync HWDGE ring) lead; x2/W2 are held back by an
  explicit dep so they never steal HBM bandwidth from the x1 stream.
- Warm-up matmuls on junk data run during the initial DMA fill so the
  PE's HAM activity window flips to full clock before real work lands.
- x2 is pre-transposed on the host to [d2-partition, l2-free] so the
  per-batch mean is a free-dim reduction, done on the otherwise-idle
  Scalar engine (activation copy with accum_out). This removes all
  partition-reduce matmuls from the PE stream.
- c[b] = xbar2 @ W2.T for BOTH batches in one 12-matmul group (the two
  means are the 2 columns of the stationary operand), inserted mid-way
  through group 2 where its inputs are long ready.
- Output groups 0-2 evacuate PSUM as plain copies (bias deferred until
  c is ready); the deferred bias adds + stores are interleaved into
  group 3's emission so the DVE never head-blocks the PSUM evacuations.

Host-side prep (layout/dtype only): partition-major SBUF images, bf16
casts (rel err ~3e-3 vs the 2e-2 gate), output stored bf16 and upcast
on the host.
"""

import sys

import numpy as np

# concourse normally comes from the axon site overlay already on sys.path;
# append /opt/trn_rl_repo as a fallback only.
if "/opt/trn_rl_repo" not in sys.path:
    sys.path.append("/opt/trn_rl_repo")

N_CORES = 8
B_PER_CORE = 2
L = 2048
D = 768  # d1 == d2 == d3 == 768
P = 128
NCH = D // P  # 6 contraction chunks
M = B_PER_CORE * L  # 4096 rows per core
NQ = 16  # x1 quarter-group DMAs, 2 m-tiles each
TPG = 8  # m-tiles per output group
NGRP = (M // P) // TPG  # 4 groups
SPT = 4  # m-tiles per output store
NST = (M // P) // SPT  # 8 stores


def build_nc(debug=False, explicit_ldw=False):
    import concourse.bacc as bacc
    import concourse.mybir as mybir
    import concourse.tile as tile

    f32 = mybir.dt.float32
    bf16 = mybir.dt.bfloat16
    fp8 = mybir.dt.float8e4
    add = mybir.AluOpType.add
    Copy = mybir.ActivationFunctionType.Copy

    nc = bacc.Bacc(None, target_bir_lowering=False, debug=debug)

    # x2 and W2 only feed the mean term c[b] (magnitude ~5% of the output,
    # averaged over 2048 rows), so fp8 quantization error lands ~1e-3 on the
    # output while halving their HBM traffic - the load phase is what gates
    # the c-path readiness.
    x1h = nc.declare_dram_parameter("x1h", [NQ, P, NCH, 2 * P], bf16, isOutput=False)
    x2h = nc.declare_dram_parameter("x2h", [B_PER_CORE, P, NCH, L], fp8, isOutput=False)
    w1h = nc.declare_dram_parameter("w1h", [P, NCH, D], bf16, isOutput=False)
    w2h = nc.declare_dram_parameter("w2h", [P, NCH, D], fp8, isOutput=False)
    bsh = nc.declare_dram_parameter("bsh", [B_PER_CORE, D], f32, isOutput=False)
    outh = nc.declare_dram_parameter("outh", [NST, P, SPT, D], bf16, isOutput=True)

    with tile.TileContext(nc) as tc:
        with (
            tc.tile_pool(name="const", bufs=1) as const,
            tc.tile_pool(name="x1p", bufs=1) as x1p,
            tc.tile_pool(name="x2p", bufs=1) as x2p,
            tc.tile_pool(name="yp", bufs=1) as yp,
            tc.tile_pool(name="psY", bufs=3, space="PSUM") as psY,
            tc.tile_pool(name="psC", bufs=1, space="PSUM") as psC,
        ):
            # ---- early constants ----
            warm = const.tile([P, 512], bf16)
            nc.vector.memset(warm[:], 0.03125)
            w1sb = const.tile([P, NCH, D], bf16)
            # W1 in 3 chunk pairs so the first matmuls' weights land first
            nc.scalar.dma_start(w1sb[:, 0:2, :], w1h[:, 0:2, :])
            nc.scalar.dma_start(w1sb[:, 2:4, :], w1h[:, 2:4, :])
            nc.scalar.dma_start(w1sb[:, 4:6, :], w1h[:, 4:6, :])

            # ---- x1 quarter-group loads (Sync ring), 3 in flight ----
            # The SDMA engines round-robin over ALL queued descriptors at
            # packet granularity, so an unchained backlog makes every
            # transfer finish near the end together; a depth-3 chain keeps
            # completions incremental at full ring throughput.
            x1tiles = []
            x1dmas = []
            for q in range(NQ):
                xq = x1p.tile([P, NCH, 2 * P], bf16, tag=f"x1q{q}")
                x1tiles.append(xq)
                dma = nc.sync.dma_start(xq[:], x1h[q])
                if q >= 3:
                    tile.add_dep_helper(
                        dma.ins, x1dmas[q - 3].ins, sync=True,
                        reason="x1 JIT pacing",
                    )
                x1dmas.append(dma)

            # ---- x2 chunk loads + mean, both on the Scalar engine ----
            # xbtf[p, c, b] = sum_j x2[b, j, c*128+p] via activation with
            # accum_out. Emission interleaves [reduce chunk i, dma chunk
            # i+3]: each 2.2us reduce paces the next chunk dispatch, so ~3
            # chunks stay in flight with no sequencer-stalling dep chain,
            # and the reduces start as soon as the first chunk lands.
            x2tiles = []
            x2dmas = []
            bsum_sb = const.tile([B_PER_CORE, D], f32)
            for b in range(B_PER_CORE):
                xt = x2p.tile([P, NCH, L], fp8, tag=f"x2t{b}")
                x2tiles.append(xt)

            def x2_dma(i):
                b, c = i // NCH, i % NCH
                dma = nc.scalar.dma_start(
                    x2tiles[b][:, c, :], x2h[b, :, c, :]
                )
                if i == 0:
                    # x2 yields HBM bandwidth until x1 quarter 0 has landed
                    tile.add_dep_helper(
                        dma.ins, x1dmas[0].ins, sync=True,
                        reason="x2 yields to x1 q0",
                    )
                x2dmas.append(dma)

            xbtf = const.tile([P, NCH, B_PER_CORE], f32)
            scr = const.tile([P, L], bf16)
            for i in range(3):
                x2_dma(i)
            for i in range(B_PER_CORE * NCH):
                b, c = i // NCH, i % NCH
                nc.scalar.activation(
                    scr[:], x2tiles[b][:, c, :], Copy,
                    accum_out=xbtf[:, c, b : b + 1],
                )
                if i + 3 < B_PER_CORE * NCH:
                    x2_dma(i + 3)
                if i == NCH - 1:
                    nc.scalar.dma_start(bsum_sb[:], bsh[:])
                    w2sb = const.tile([P, NCH, D], fp8)
                    nc.scalar.dma_start(w2sb[:], w2h[:])

            # ---- warm-up matmuls: flip the HAM clock gate during DMA fill ----
            pc = psC.tile([1, D], f32)
            for _ in range(3):
                nc.tensor.matmul(
                    pc[:, 0:512], warm[:, 0:1], warm[:], start=True, stop=True,
                )

            # ---- main matmul stream ----
            cr = [None, None]

            def emit_mtile(t, ys, fused_b=None):
                q, sub = t // 2, t % 2
                xq = x1tiles[q]
                py_ = psY.tile([P, D], f32)
                for c in range(NCH):
                    xw = xq[:, c, sub * P : (sub + 1) * P]
                    nc.tensor.matmul(
                        py_[:, 0:512], xw, w1sb[:, c, 0:512],
                        start=(c == 0), stop=(c == NCH - 1),
                    )
                    nc.tensor.matmul(
                        py_[:, 512:768], xw, w1sb[:, c, 512:768],
                        start=(c == 0), stop=(c == NCH - 1),
                    )
                tl = t % TPG
                if fused_b is None:
                    nc.vector.tensor_copy(ys[:, tl, :], py_[:])
                else:
                    nc.vector.tensor_tensor(
                        ys[:, tl, :], py_[:], cr[fused_b][:], op=add
                    )

            def group_tiles(g):
                ys = yp.tile([P, TPG, D], bf16, tag=f"ys{g}")
                return ys

            ys_g = [group_tiles(g) for g in range(NGRP)]

            # t0..t15: deferred bias (c is not ready yet)
            with nc.named_scope("grp01"):
                for t in range(0, 16):
                    emit_mtile(t, ys_g[t // TPG])

            # ---- c path: c[b] = mean(x2[b]) @ W2.T + (b1 + b2) ----
            # Two M=1 matmul passes sharing one PSUM tile, so both results
            # land on partition 0 where partition_broadcast can source
            # them directly (an SBUF->SBUF hop for partition 1 costs ~4us
            # of serial DMA latency). Batch 1's pass runs right after t15
            # so cr1 is ready when t16's fused evacuation is due; batch
            # 0's pass is deferred past t17 to spread the PE cost.
            xbtb = const.tile([P, NCH, B_PER_CORE], bf16)

            def c_lin(b):
                for c in range(NCH):
                    nc.tensor.matmul(
                        pc[:, 0:512], xbtb[:, c, b : b + 1],
                        w2sb[:, c, 0:512],
                        start=(c == 0), stop=(c == NCH - 1),
                    )
                for c in range(NCH):
                    nc.tensor.matmul(
                        pc[:, 512:768], xbtb[:, c, b : b + 1],
                        w2sb[:, c, 512:768],
                        start=(c == 0), stop=(c == NCH - 1),
                    )
                csb = const.tile([1, D], bf16, tag=f"cs{b}")
                nc.vector.tensor_tensor(csb[:], pc[:], bsum_sb[0:1, :], op=add)
                crb = const.tile([P, D], bf16, tag=f"cr{b}")
                nc.gpsimd.partition_broadcast(crb[:], csb[:])
                cr[b] = crb

            with nc.named_scope("grp2a"):
                for t in range(16, 20):
                    emit_mtile(t, ys_g[2])

            with nc.named_scope("c_path"):
                # mean scale + bf16 cast on idle GpSimd, off the DVE FIFO
                nc.gpsimd.tensor_scalar_mul(xbtb[:], xbtf[:], 1.0 / L)
                c_lin(1)
                c_lin(0)

            # deferred bias + stores for t0..t19, interleaved into the
            # t20..t31 emission at <=2 per m-tile so the DVE keeps pace
            # with the PSUM evacuations (evac 0.9us + 2 adds ~1us vs the
            # 1.95us m-tile period)
            flush_items = []
            for tl in range(4):  # t16..t19 first: ready earliest (cr1)
                flush_items.append(("add", 2, 1, tl))
            flush_items.append(("store", 2, 3))
            for g in (1, 0):
                for tl in range(TPG):
                    flush_items.append(("add", g, 0, tl))
                    if tl % SPT == SPT - 1:
                        flush_items.append(("store", g, tl))

            def emit_flush(item):
                if item[0] == "add":
                    _, g, b, tl = item
                    nc.vector.tensor_tensor(
                        ys_g[g][:, tl, :], ys_g[g][:, tl, :], cr[b][:], op=add
                    )
                else:
                    # deferred stores drain on the Scalar ring, which is
                    # idle after the loads - the Sync ring keeps the
                    # fused-path stores, halving the store-drain time
                    _, g, tl = item
                    st = g * 2 + tl // SPT
                    ring = nc.sync if g == 2 else nc.scalar
                    ring.dma_start(
                        outh[st], ys_g[g][:, tl - SPT + 1 : tl + 1, :]
                    )

            slot_quota = {t: 2 for t in range(20, 32)}
            fi = 0
            with nc.named_scope("grp23b"):
                for t in range(20, 32):
                    g = t // TPG
                    emit_mtile(t, ys_g[g], fused_b=1)
                    tl = t % TPG
                    if t < 28:
                        if tl % SPT == SPT - 1:
                            nc.sync.dma_start(
                                outh[t // SPT],
                                ys_g[g][:, tl - SPT + 1 : tl + 1, :],
                            )
                    elif t % 2 == 1:
                        # final tiles: pairwise stores so the tail DMA is small
                        lo = (tl % SPT) - 1
                        nc.sync.dma_start(
                            outh[7][:, lo : lo + 2, :],
                            ys_g[3][:, tl - 1 : tl + 1, :],
                        )
                    for _ in range(slot_quota.get(t, 0)):
                        if fi < len(flush_items):
                            emit_flush(flush_items[fi])
                            fi += 1
                while fi < len(flush_items):
                    emit_flush(flush_items[fi])
                    fi += 1

    return nc


def make_in_maps(x1, x2, W1, b1, W2, b2):
    import ml_dtypes

    bf16 = ml_dtypes.bfloat16
    fp8 = ml_dtypes.float8_e4m3fn

    def wlayout(W, dt):
        # [e, d] -> W.T [d, e] -> [p, c, e] with d = c*128 + p
        wt = np.ascontiguousarray(W.T).reshape(NCH, P, D).transpose(1, 0, 2)
        return np.ascontiguousarray(wt).astype(dt)

    w1h = wlayout(W1, bf16)
    w2h = wlayout(W2, fp8)
    bsh = np.ascontiguousarray(
        np.broadcast_to((b1 + b2).reshape(1, D), (B_PER_CORE, D))
    ).astype(np.float32)
    in_maps = []
    for k in range(N_CORES):
        x1_s = x1[k * B_PER_CORE : (k + 1) * B_PER_CORE]  # [2, 2048, 768]
        x2_s = x2[k * B_PER_CORE : (k + 1) * B_PER_CORE]
        # x1t [d, m] with col m = b*2048 + i, then quarter-major image
        x1t = np.transpose(x1_s, (2, 0, 1)).reshape(D, M)
        x1h = np.ascontiguousarray(
            x1t.reshape(NCH, P, NQ, 2 * P).transpose(2, 1, 0, 3)
        ).astype(bf16)  # [q, p, c, m_in_quarter]
        # x2 transposed: [b, p, c, j] with d = c*128 + p
        x2h = np.ascontiguousarray(
            np.transpose(x2_s, (0, 2, 1)).reshape(B_PER_CORE, NCH, P, L)
            .transpose(0, 2, 1, 3)
        ).astype(fp8)
        in_maps.append(
            {"x1h": x1h, "x2h": x2h, "w1h": w1h, "w2h": w2h, "bsh": bsh}
        )
    return in_maps


def kernel(x1, x2, W1, b1, W2, b2, trace=False, explicit_ldw=False):
    from concourse.bass_utils import run_bass_kernel_spmd

    # accept jax arrays / lists transparently
    x1, x2, W1, b1, W2, b2 = (
        np.asarray(t, dtype=np.float32) for t in (x1, x2, W1, b1, W2, b2)
    )
    nc = build_nc(debug=False, explicit_ldw=explicit_ldw)
    nc.finalize()
    in_maps = make_in_maps(x1, x2, W1, b1, W2, b2)
    res = run_bass_kernel_spmd(
        nc, in_maps, core_ids=list(range(N_CORES)), trace=trace
    )
    shards = []
    for k in range(N_CORES):
        oh = res.results[k]["outh"]  # [NST, P, SPT, D] bf16, row = (s*SPT+t)*128+p
        flat = (
            oh.astype(np.float32).transpose(0, 2, 1, 3).reshape(M, D)
        )
        shards.append(flat.reshape(B_PER_CORE, L, D))
    out = np.concatenate(shards, axis=0)
    if trace:
        kernel.last_result = res
    return out
